# revision 9
# baseline (speedup 1.0000x reference)
"""KNN grouping kernel (PointNet++ style) for Trainium2, 8 NeuronCores.

Problem: B=4 batches, N=8192 source points, M=2048 query points, C=64
feature channels, K=16 nearest neighbors.  Output [B, 3+C, M, K].

Sharding: 8 cores = (4 batches) x (2 halves of M).  Each core handles one
batch and 1024 queries against the full N=8192 source set.

Per-core algorithm:
  1. TensorE: score[m, n] = 2*q_m.p_n - |p_n|^2  (monotone in -distance)
     via one augmented matmul  lhsT=[2q; 1] (4 x 128), rhs=[p; -|p|^2] (4 x N).
  2. DVE:  top-16 of each score row with max / max_index / match_replace.
  3. Replicate the 16 index columns x5 (free-dim doubling copies), then
     DMA-transpose the [128, 128] u16 tile into gpsimd "wrapped" layout.
  4. GPSIMD ap_gather: one 80-channel gather pulls features (rows 0-63) and
     points (rows 64-66); a second 16-channel gather broadcasts query coords.
  5. Subtract query coords from gathered points (recentering), DMA out.

SBUF partition layout honours the start-partition rule (0/32/64/96 only).
"""

import numpy as np
from contextlib import ExitStack

import concourse.bacc as bacc
import concourse.tile as tile
import concourse.mybir as mybir
from concourse import bass
from concourse.bass_utils import run_bass_kernel_spmd

B, N, M, C, K = 4, 8192, 2048, 64, 16
MH = M // 2          # 1024 queries per core
NT = MH // 128       # 8 query tiles per core
CH = 80              # gather channels: 0-63 feats, 64-66 pts, 67-79 pad
F32 = mybir.dt.float32
U16 = mybir.dt.uint16
I16 = mybir.dt.int16

NEG_BIG = -1.0e30


def build_program():
    nc = bacc.Bacc("TRN2", target_bir_lowering=False, debug=False, num_devices=8)

    lhsT_d = nc.dram_tensor("lhsT", [4, MH], F32, kind="ExternalInput")
    rhs_d = nc.dram_tensor("rhs", [4, N], F32, kind="ExternalInput")
    qbc_d = nc.dram_tensor("qbc", [3, MH * K], F32, kind="ExternalInput")
    gf_d = nc.dram_tensor("gfeat", [67, N], F32, kind="ExternalInput")
    out_d = nc.dram_tensor("out", [67, MH, K], F32, kind="ExternalOutput")

    with tile.TileContext(nc) as tc, ExitStack() as ctx:
        const = ctx.enter_context(tc.tile_pool(name="const", bufs=1))
        sc_pool = ctx.enter_context(tc.tile_pool(name="scores", bufs=2))
        ps_pool = ctx.enter_context(tc.tile_pool(name="psum", bufs=2, space="PSUM"))
        g_pool = ctx.enter_context(tc.tile_pool(name="g", bufs=3))
        idx_pool = ctx.enter_context(tc.tile_pool(name="idx", bufs=2))
        v_pool = ctx.enter_context(tc.tile_pool(name="v", bufs=4))

        # ---- one-time loads ----
        # gsrc rows: 0-63 features, 64-66 points, 67-79 pad
        gsrc = const.tile([CH, N], F32)
        nc.vector.memset(gsrc[64:CH, :], 0.0)
        nc.sync.dma_start(out=gsrc[0:64, :], in_=gf_d[0:64, :])
        nc.sync.dma_start(out=gsrc[64:67, :], in_=gf_d[64:67, :])

        rhs_sb = const.tile([4, N], F32)
        nc.sync.dma_start(out=rhs_sb[:], in_=rhs_d[:])
        lhsT_sb = const.tile([4, MH], F32)
        nc.sync.dma_start(out=lhsT_sb[:], in_=lhsT_d[:])

        for t in range(NT):
            # ---- scores via matmul ----
            scores = sc_pool.tile([128, N], F32)
            lhsT_t = lhsT_sb[:, t * 128:(t + 1) * 128]
            for cchunk in range(4):
                psum = ps_pool.tile([128, 2048], F32)
                for qq in range(4):
                    col0 = cchunk * 2048 + qq * 512
                    nc.tensor.matmul(
                        psum[:, qq * 512:(qq + 1) * 512],
                        lhsT_t,
                        rhs_sb[:, col0:col0 + 512],
                        start=True,
                        stop=True,
                    )
                nc.scalar.copy(
                    scores[:, cchunk * 2048:(cchunk + 1) * 2048], psum[:]
                )

            # ---- top-16 (DVE) ----
            idx128 = idx_pool.tile([128, 128], U16, tag="idx128")
            v8a = v_pool.tile([128, 8], F32, tag="v8a")
            v8b = v_pool.tile([128, 8], F32, tag="v8b")
            nc.vector.max(v8a[:], scores[:])
            nc.vector.max_index(idx128[:, 0:8], v8a[:], scores[:])
            nc.vector.match_replace(scores[:], v8a[:], scores[:], NEG_BIG)
            nc.vector.max(v8b[:], scores[:])
            nc.vector.max_index(idx128[:, 8:16], v8b[:], scores[:])

            # replicate the 16 index columns into all 128 columns (x8)
            nc.vector.tensor_copy(idx128[:, 16:32], idx128[:, 0:16])
            nc.vector.tensor_copy(idx128[:, 32:64], idx128[:, 0:32])
            nc.vector.tensor_copy(idx128[:, 64:128], idx128[:, 0:64])

            # ---- transpose indices to wrapped gpsimd layout ----
            idxt = idx_pool.tile([128, 128], U16, tag="idxt")
            nc.sync.dma_start(out=idxt[:], in_=idx128[:], transpose=True)

            # ---- gather feats+points in one op; query-broadcast in another ----
            g = g_pool.tile([CH, 128 * K], F32, tag="g")
            nc.gpsimd.ap_gather(
                g[:], gsrc[:], idxt[0:CH, :].bitcast(I16),
                channels=CH, num_elems=N, d=1, num_idxs=128 * K,
            )
            # k-broadcast query coords (host-precomputed), same base partition
            qrep = g_pool.tile([80, 128 * K], F32, tag="qrep")
            nc.sync.dma_start(
                out=qrep[64:67, :],
                in_=qbc_d[:, t * 128 * K:(t + 1) * 128 * K],
            )

            # ---- recenter gathered points ----
            nc.vector.tensor_sub(g[64:67, :], g[64:67, :], qrep[64:67, :])

            # ---- write out: rows 64-66 -> out channels 0-2, rows 0-63 -> 3-67
            nc.sync.dma_start(
                out=out_d[0:3, t * 128:(t + 1) * 128, :],
                in_=g[64:67, :].rearrange("p (m k) -> p m k", k=K),
            )
            nc.sync.dma_start(
                out=out_d[3:67, t * 128:(t + 1) * 128, :],
                in_=g[0:64, :].rearrange("p (m k) -> p m k", k=K),
            )

    nc.compile()
    return nc


_NC_CACHE = {}


def _get_nc():
    if "nc" not in _NC_CACHE:
        _NC_CACHE["nc"] = build_program()
    return _NC_CACHE["nc"]


def make_in_maps(points, new_points, features):
    in_maps = []
    for c in range(8):
        b, h = divmod(c, 2)
        p = np.asarray(points[b], dtype=np.float32)          # [3, N]
        q = np.asarray(new_points[b][:, h * MH:(h + 1) * MH], dtype=np.float32)
        pp = (p * p).sum(axis=0)                              # [N]
        lhsT = np.concatenate([2.0 * q, np.ones((1, MH), np.float32)], axis=0)
        rhs = np.concatenate([p, -pp[None, :]], axis=0)
        gfeat = np.concatenate(
            [np.asarray(features[b], dtype=np.float32), p], axis=0
        )                                                     # [67, N] feats;pts
        qbc = np.repeat(q, K, axis=1)                         # [3, MH*K]
        in_maps.append({
            "lhsT": np.ascontiguousarray(lhsT),
            "rhs": np.ascontiguousarray(rhs),
            "qbc": np.ascontiguousarray(qbc),
            "gfeat": np.ascontiguousarray(gfeat),
        })
    return in_maps


def assemble(results):
    out = np.empty((B, 3 + C, M, K), np.float32)
    for c in range(8):
        b, h = divmod(c, 2)
        out[b, :, h * MH:(h + 1) * MH, :] = results[c]["out"]
    return out


def kernel(points, new_points, features, _trace=False, _tmpdir=None):
    nc = _get_nc()
    in_maps = make_in_maps(points, new_points, features)
    res = run_bass_kernel_spmd(
        nc, in_maps, list(range(8)), trace=_trace, tmpdir=_tmpdir
    )
    out = assemble(res.results)
    if _trace:
        return out, res
    return out
